# revision 8
# baseline (speedup 1.0000x reference)
"""Trainium2 Bass kernel for nn_DiscUpdateUnit (bipartite GNN message passing).

Math (per reference):
    msg_u   = segment_sum(vals * (xi @ W_iu.T)[i_idx], u_idx)
            = segment_sum(vals * xi[i_idx], u_idx) @ W_iu.T        (linearity)
    delta_u = relu(xu @ W_uu.T + b_uu + msg_u) * (segsum(vals, u_idx) > 0)
    xu_plus = xu + delta_u     (and the symmetric item update)
    loss    = sum(delta_u^2)/sum(mask_u) + sum(delta_i^2)/sum(mask_i)

Mapping: destination sharding across 8 NeuronCores (no collectives).  Per core
and phase the host sorts its edges by (source-quarter, dest window); 128-edge
blocks are fetched with dma_gather from bf16 node tables (edges land on
partitions), and the segment-sum runs on TensorE:
    psum[feat, dest_window] += G_block^T @ S_block
with S a host-built sparse [128 x width] bf16 tile (vals embedded; width 128
with start=True for the first block of each (quarter, window), else a 32-wide
32-aligned band).  Degree rowsums come from ones^T @ S into a [1,128] psum row.
The epilogue (weight matmuls, bias+relu on ScalarE, degree mask via a rank-1
replicate matmul, residual add, loss reduction) stays on-chip in feat-major
layout; the host transposes outputs back and combines scalar loss partials.
"""
import math
import os
from collections import defaultdict, deque

import numpy as np
import ml_dtypes

import concourse.mybir as mybir
import concourse.tile as tile
from concourse import bacc
from concourse.bass_utils import run_bass_kernel_spmd
from concourse.library_config import mlp as mlp_lib

F32 = mybir.dt.float32
BF16 = mybir.dt.bfloat16
I16 = mybir.dt.int16

NCORES = 8
WIN = 128          # dest window = psum tile width
SUB = 32           # banded block width (32-aligned within window)
CHUNK_BLOCKS = int(__import__('os').environ.get('K_CHUNK', '64'))  # max blocks per dma_gather call


# ----------------------------------------------------------------------------
# host-side planning
# ----------------------------------------------------------------------------

def _cut_blocks(d_l, e0, e1):
    """Cut dest-sorted edge range [e0,e1) into blocks of <=128 edges.
    First block is full-window (any span); later blocks stay inside one
    32-aligned dest sub-range.  Returns [(estart, eend, sub_or_minus1)]."""
    out = []
    i = e0
    first = True
    while i < e1:
        if first:
            j = min(i + 128, e1)
            out.append((i, j, -1))
        else:
            base = int(d_l[i] // SUB) * SUB
            j = min(i + 128, int(np.searchsorted(d_l[e0:e1], base + SUB) + e0))
            out.append((i, j, (base % WIN) // SUB))
        i = j
        first = False
    return out


class PhasePlan:
    """Uniform (cross-core) block schedule + per-core edge grouping."""

    def __init__(self, dest, src, vals, n_dest, n_src, q_rows):
        assert n_dest % NCORES == 0
        self.dshard = n_dest // NCORES
        self.nwin = math.ceil(self.dshard / WIN)
        self.dpad = self.nwin * WIN
        self.nq = math.ceil(n_src / q_rows)
        self.q_rows = q_rows
        nsub = WIN // SUB
        owner = dest // self.dshard

        counts_full = np.zeros((self.nq, self.nwin), np.int64)
        counts_band = np.zeros((self.nq, self.nwin, nsub), np.int64)
        self.percore = []  # (d_l, s_l, v_l, blocks{(q,w): [(e0,e1,sub)]})
        for c in range(NCORES):
            m = owner == c
            d_l = (dest[m] - c * self.dshard).astype(np.int64)
            s_l = src[m].astype(np.int64)
            v_l = np.asarray(vals)[m].astype(np.float32)
            q_l = s_l // q_rows
            w_l = d_l // WIN
            order = np.lexsort((d_l, w_l, q_l))
            d_l, s_l, v_l, q_l, w_l = (a[order] for a in (d_l, s_l, v_l, q_l, w_l))
            s_l = s_l - q_l * q_rows
            key = q_l * self.nwin + w_l
            bounds = np.searchsorted(key, np.arange(self.nq * self.nwin + 1))
            blocks = {}
            for q in range(self.nq):
                for w in range(self.nwin):
                    k = q * self.nwin + w
                    e0, e1 = int(bounds[k]), int(bounds[k + 1])
                    if e0 == e1:
                        blocks[(q, w)] = []
                        continue
                    bl = _cut_blocks(d_l, e0, e1)
                    blocks[(q, w)] = bl
                    nf = sum(1 for b in bl if b[2] < 0)
                    counts_full[q, w] = max(counts_full[q, w], nf)
                    sub = np.zeros(nsub, np.int64)
                    for b in bl:
                        if b[2] >= 0:
                            sub[b[2]] += 1
                    counts_band[q, w] = np.maximum(counts_band[q, w], sub)
            self.percore.append((d_l, s_l, v_l, blocks))
        counts_full[0, :] = np.maximum(counts_full[0, :], 1)
        self.counts_full = counts_full
        self.counts_band = counts_band

        # global slot schedule (q, w, width, base_rel), chunks cut at (q,w) bounds
        self.slot_meta = []  # (chunk, slot_in_chunk, s_off_in_chunk, q, w, width, base_rel, first, last)
        self.chunks = []     # (q, idx_col_off, s_col_off, nblocks, s_cols)
        icol = scol = 0
        for q in range(self.nq):
            qslots = []
            for w in range(self.nwin):
                n = int(counts_full[q, w])
                total = n + int(counts_band[q, w].sum())
                k = 0
                for _ in range(n):
                    qslots.append((w, WIN, 0, k == 0, k == total - 1)); k += 1
                for s in range(nsub):
                    for _ in range(int(counts_band[q, w, s])):
                        qslots.append((w, SUB, s * SUB, k == 0, k == total - 1)); k += 1
            for c0 in range(0, len(qslots), CHUNK_BLOCKS):
                cur = qslots[c0:c0 + CHUNK_BLOCKS]
                ck = len(self.chunks)
                soff = 0
                for sl, (w, width, brel, fst, lst) in enumerate(cur):
                    self.slot_meta.append((ck, sl, soff, q, w, width, brel, fst, lst))
                    soff += width
                self.chunks.append((q, icol, scol, len(cur), soff))
                icol += len(cur) * 8
                scol += soff
        self.idx_cols = icol
        self.s_cols = scol
        self.nslots = len(self.slot_meta)
        self.max_chunk_blocks = max(ch[3] for ch in self.chunks)
        self.max_chunk_scols = max(ch[4] for ch in self.chunks)
        # first quarter that visits each window (for copy-vs-add evacuation)
        self.first_q = np.zeros(self.nwin, np.int64)
        for w in range(self.nwin):
            for q in range(self.nq):
                if counts_full[q, w] + counts_band[q, w].sum() > 0:
                    self.first_q[w] = q
                    break

    def core_streams(self, c):
        """idx [128, nslots*8] i16 (wrapped+replicated) and S [128, s_cols] bf16."""
        d_l, s_l, v_l, blocks = self.percore[c]
        idx = np.zeros((16, self.nslots * 8), np.int16)
        sarr = np.zeros((128, self.s_cols), np.float32)
        pool = defaultdict(deque)
        for (q, w), bl in blocks.items():
            for (e0, e1, sub) in bl:
                pool[(q, w, sub)].append((e0, e1))
        for (ck, sl, soff, q, w, width, brel, fst, lst) in self.slot_meta:
            kind = -1 if width == WIN else brel // SUB
            dq = pool.get((q, w, kind))
            if not dq:
                continue
            e0, e1 = dq.popleft()
            n = e1 - e0
            pos = np.arange(n)
            icol0 = self.chunks[ck][1] + sl * 8
            idx[pos % 16, icol0 + pos // 16] = s_l[e0:e1].astype(np.int16)
            scol0 = self.chunks[ck][2] + soff
            cols = d_l[e0:e1] - (w * WIN + brel)
            assert (cols >= 0).all() and (cols < width).all()
            sarr[pos % 128, scol0 + cols] = v_l[e0:e1]
        for left in pool.values():
            assert not left, "unconsumed blocks: padding bug"
        return np.tile(idx, (8, 1)), sarr.astype(ml_dtypes.bfloat16)


# ----------------------------------------------------------------------------
# device program
# ----------------------------------------------------------------------------

def build_program(plan_u, plan_i, d):
    nu_pad, ni_pad = plan_u.dpad, plan_i.dpad
    n_i_rows = plan_u.nq * plan_u.q_rows
    n_u_rows = plan_i.nq * plan_i.q_rows

    nc = bacc.Bacc("TRN2", target_bir_lowering=False, debug=False)
    t = {}
    t["xi_bf"] = nc.dram_tensor("xi_bf", [n_i_rows, d], BF16, kind="ExternalInput")
    t["xu_bf"] = nc.dram_tensor("xu_bf", [n_u_rows, d], BF16, kind="ExternalInput")
    t["xuT"] = nc.dram_tensor("xuT", [d, nu_pad], F32, kind="ExternalInput")
    t["xiT"] = nc.dram_tensor("xiT", [d, ni_pad], F32, kind="ExternalInput")
    t["idx_u"] = nc.dram_tensor("idx_u", [128, plan_u.nslots * 8], I16, kind="ExternalInput")
    t["idx_i"] = nc.dram_tensor("idx_i", [128, plan_i.nslots * 8], I16, kind="ExternalInput")
    t["s_u"] = nc.dram_tensor("s_u", [128, plan_u.s_cols], BF16, kind="ExternalInput")
    t["s_i"] = nc.dram_tensor("s_i", [128, plan_i.s_cols], BF16, kind="ExternalInput")
    for nm in ("w_uu_t", "w_iu_t", "w_ii_t", "w_ui_t"):
        t[nm] = nc.dram_tensor(nm, [d, d], BF16, kind="ExternalInput")
    t["b_u"] = nc.dram_tensor("b_u", [d, 1], F32, kind="ExternalInput")
    t["b_i"] = nc.dram_tensor("b_i", [d, 1], F32, kind="ExternalInput")
    t["xuT_plus"] = nc.dram_tensor("xuT_plus", [d, nu_pad], F32, kind="ExternalOutput")
    t["xiT_plus"] = nc.dram_tensor("xiT_plus", [d, ni_pad], F32, kind="ExternalOutput")
    t["loss_parts"] = nc.dram_tensor("loss_parts", [1, 4], F32, kind="ExternalOutput")

    gmax = max(plan_u.max_chunk_blocks, plan_i.max_chunk_blocks)
    smax = max(plan_u.max_chunk_scols, plan_i.max_chunk_scols)

    with tile.TileContext(nc) as tc:
        with (
            tc.tile_pool(name="const", bufs=1) as constp,
            tc.tile_pool(name="acc", bufs=1) as accp,
            tc.tile_pool(name="io", bufs=3) as iop,
            tc.tile_pool(name="gring", bufs=2) as gringp,
            tc.tile_pool(name="sring", bufs=2) as sringp,
            tc.tile_pool(name="iring", bufs=2) as iringp,
            tc.tile_pool(name="ps_s", bufs=2, space="PSUM") as ps_sp,
            tc.tile_pool(name="ps_r", bufs=2, space="PSUM") as ps_rp,
            tc.tile_pool(name="ps_e", bufs=2, space="PSUM") as ps_ep,
            tc.tile_pool(name="ps_m", bufs=2, space="PSUM") as ps_mp,
        ):
            nc.gpsimd.load_library(mlp_lib)

            ones_col = constp.tile([128, 1], BF16)
            ones_row = constp.tile([1, 128], BF16)
            ones_f32 = constp.tile([128, 1], F32)
            nc.vector.memset(ones_col[:], 1.0)
            nc.vector.memset(ones_row[:], 1.0)
            nc.vector.memset(ones_f32[:], 1.0)
            w_t = {}
            for nm in ("w_uu_t", "w_iu_t", "w_ii_t", "w_ui_t"):
                w_t[nm] = constp.tile([d, d], BF16, name=nm, tag=nm)
                nc.sync.dma_start(w_t[nm][:], t[nm].ap())
            b_t = {}
            for nm in ("b_u", "b_i"):
                b_t[nm] = constp.tile([d, 1], F32, name=nm, tag=nm)
                nc.sync.dma_start(b_t[nm][:], t[nm].ap())

            loss_t = accp.tile([1, 4], F32)
            nc.vector.memset(loss_t[:], 0.0)

            def message_pass(plan, table, idx_dram, s_dram, sT_t, mask_t):
                si = 0
                for (q, icol, scol, nb, scols) in plan.chunks:
                    idx_t = iringp.tile([128, gmax * 8], I16, tag="idx")
                    g_t = gringp.tile([128, gmax, d], BF16, tag="g")
                    s_t = sringp.tile([128, smax], BF16, tag="s")
                    nc.sync.dma_start(idx_t[:, :nb * 8],
                                      idx_dram.ap()[:, icol:icol + nb * 8])
                    nc.sync.dma_start(s_t[:, :scols],
                                      s_dram.ap()[:, scol:scol + scols])
                    tbl = table.ap()[q * plan.q_rows:(q + 1) * plan.q_rows, :]
                    nc.gpsimd.dma_gather(g_t[:, :nb, :], tbl, idx_t[:, :nb * 8],
                                         nb * 128, nb * 128, d)
                    for j in range(si, si + nb):
                        ck, sl, soff, qq, w, width, brel, fst, lst = plan.slot_meta[j]
                        if fst:
                            ps_s = ps_sp.tile([d, WIN], F32, tag="s")
                            ps_r = ps_rp.tile([1, WIN], F32, tag="r")
                        rhs = s_t[:, soff:soff + width]
                        nc.tensor.matmul(ps_s[:, brel:brel + width],
                                         g_t[:, sl, :], rhs,
                                         start=fst, stop=lst)
                        nc.tensor.matmul(ps_r[0:1, brel:brel + width],
                                         ones_col[:], rhs,
                                         start=fst, stop=lst)
                        if lst:
                            c0 = w * WIN
                            if qq == plan.first_q[w]:
                                nc.vector.tensor_copy(sT_t[:, c0:c0 + WIN], ps_s[:])
                                nc.vector.tensor_scalar(
                                    mask_t[0:1, c0:c0 + WIN], ps_r[:],
                                    0.0, None, mybir.AluOpType.is_gt)
                            else:
                                nc.vector.tensor_add(sT_t[:, c0:c0 + WIN],
                                                     sT_t[:, c0:c0 + WIN], ps_s[:])
                                mrow = iop.tile([1, WIN], BF16, tag="mrowtmp")
                                nc.vector.tensor_scalar(
                                    mrow[:], ps_r[:], 0.0, None,
                                    mybir.AluOpType.is_gt)
                                nc.vector.tensor_tensor(
                                    mask_t[0:1, c0:c0 + WIN],
                                    mask_t[0:1, c0:c0 + WIN], mrow[:],
                                    mybir.AluOpType.max)
                    si += nb

            def epilogue(plan, xT_dram, outT_dram, w_self, w_cross, bias,
                         sT_t, mask_t, acc_sq):
                for w in range(plan.nwin):
                    c0 = w * WIN
                    xwin = iop.tile([d, WIN], F32, tag="xwin")
                    nc.sync.dma_start(xwin[:], xT_dram.ap()[:, c0:c0 + WIN])
                    xwin_bf = iop.tile([d, WIN], BF16, tag="xwinbf")
                    nc.vector.tensor_copy(xwin_bf[:], xwin[:])
                    swin_bf = iop.tile([d, WIN], BF16, tag="swinbf")
                    nc.vector.tensor_copy(swin_bf[:], sT_t[:, c0:c0 + WIN])
                    ps_z = ps_ep.tile([d, WIN], F32, tag="e")
                    nc.tensor.matmul(ps_z[:], w_self[:], xwin_bf[:],
                                     start=True, stop=False)
                    nc.tensor.matmul(ps_z[:], w_cross[:], swin_bf[:],
                                     start=False, stop=True)
                    delta = iop.tile([d, WIN], F32, tag="delta")
                    nc.scalar.activation(delta[:], ps_z[:],
                                         mybir.ActivationFunctionType.Relu,
                                         bias=bias[:])
                    ps_m = ps_mp.tile([d, WIN], F32, tag="m")
                    nc.tensor.matmul(ps_m[:], ones_row[:],
                                     mask_t[0:1, c0:c0 + WIN],
                                     start=True, stop=True)
                    dm = iop.tile([d, WIN], F32, tag="dm")
                    nc.vector.tensor_mul(dm[:], delta[:], ps_m[:])
                    outw = iop.tile([d, WIN], F32, tag="outw")
                    nc.vector.tensor_add(outw[:], dm[:], xwin[:])
                    nc.sync.dma_start(outT_dram.ap()[:, c0:c0 + WIN], outw[:])
                    sq = iop.tile([d, WIN], F32, tag="sq")
                    nc.vector.tensor_mul(sq[:], dm[:], dm[:])
                    red = iop.tile([d, 1], F32, tag="red")
                    nc.vector.tensor_reduce(red[:], sq[:], mybir.AxisListType.X,
                                            mybir.AluOpType.add)
                    nc.vector.tensor_add(acc_sq[:], acc_sq[:], red[:])

            _phases = ((plan_u, t["xi_bf"], "u"), (plan_i, t["xu_bf"], "i"))
            if os.environ.get("K_ONE_PHASE") == "1":
                _phases = _phases[:1]
            for phase, (plan, table, sd) in enumerate(_phases):
              with tc.tile_pool(name=f"acc_{sd}", bufs=1) as accph:
                sT_t = accph.tile([d, plan.dpad], F32, name=f"sT_{sd}", tag=f"sT_{sd}")
                mask_t = accph.tile([1, plan.dpad], BF16, name=f"mask_{sd}", tag=f"mask_{sd}")
                acc_sq = accph.tile([d, 1], F32, name=f"accsq_{sd}", tag=f"accsq_{sd}")
                nc.vector.memset(acc_sq[:], 0.0)
                message_pass(plan, table, t[f"idx_{sd}"], t[f"s_{sd}"], sT_t, mask_t)
                if os.environ.get("K_NO_EPI") == "1":
                    outd = t["xuT_plus"] if sd == "u" else t["xiT_plus"]
                    nc.sync.dma_start(outd.ap(), sT_t[:])
                    continue
                epilogue(plan,
                         t["xuT"] if sd == "u" else t["xiT"],
                         t["xuT_plus"] if sd == "u" else t["xiT_plus"],
                         w_t["w_uu_t"] if sd == "u" else w_t["w_ii_t"],
                         w_t["w_iu_t"] if sd == "u" else w_t["w_ui_t"],
                         b_t["b_u"] if sd == "u" else b_t["b_i"],
                         sT_t, mask_t, acc_sq)
                ps_l = ps_ep.tile([d, WIN], F32, tag="e")
                nc.tensor.matmul(ps_l[0:1, 0:1], acc_sq[:], ones_f32[:],
                                 start=True, stop=True)
                nc.vector.tensor_copy(loss_t[0:1, 2 * phase:2 * phase + 1],
                                      ps_l[0:1, 0:1])
                nc.vector.tensor_reduce(loss_t[0:1, 2 * phase + 1:2 * phase + 2],
                                        mask_t[0:1, :], mybir.AxisListType.X,
                                        mybir.AluOpType.add)
            nc.sync.dma_start(t["loss_parts"].ap(), loss_t[:])

    nc.compile()
    return nc


# ----------------------------------------------------------------------------
# host orchestration
# ----------------------------------------------------------------------------

TIMINGS = {}
_CACHE = {}


def run(xu, xi, u_idx, i_idx, vals, W_uu, b_uu, W_ii, b_ii, W_iu, W_ui, q_rows):
    import time as _time
    xu = np.asarray(xu, np.float32)
    xi = np.asarray(xi, np.float32)
    u_idx = np.asarray(u_idx, np.int64)
    i_idx = np.asarray(i_idx, np.int64)
    vals = np.asarray(vals, np.float32)
    n_u, d = xu.shape
    n_i = xi.shape[0]

    _t0 = _time.time()
    plan_u = PhasePlan(u_idx, i_idx, vals, n_u, n_i, q_rows)
    plan_i = PhasePlan(i_idx, u_idx, vals, n_i, n_u, q_rows)
    TIMINGS["plan_s"] = _time.time() - _t0

    # shared (replicated) tables, padded to quarter multiples
    xi_bf = np.zeros((plan_u.nq * q_rows, d), ml_dtypes.bfloat16)
    xi_bf[:n_i] = xi.astype(ml_dtypes.bfloat16)
    xu_bf = np.zeros((plan_i.nq * q_rows, d), ml_dtypes.bfloat16)
    xu_bf[:n_u] = xu.astype(ml_dtypes.bfloat16)
    consts = dict(
        w_uu_t=np.ascontiguousarray(W_uu.T).astype(ml_dtypes.bfloat16),
        w_iu_t=np.ascontiguousarray(W_iu.T).astype(ml_dtypes.bfloat16),
        w_ii_t=np.ascontiguousarray(W_ii.T).astype(ml_dtypes.bfloat16),
        w_ui_t=np.ascontiguousarray(W_ui.T).astype(ml_dtypes.bfloat16),
        b_u=np.ascontiguousarray(np.asarray(b_uu, np.float32).reshape(d, 1)),
        b_i=np.ascontiguousarray(np.asarray(b_ii, np.float32).reshape(d, 1)),
    )

    in_maps = []
    for c in range(NCORES):
        idx_u, s_u = plan_u.core_streams(c)
        idx_i, s_i = plan_i.core_streams(c)
        xuT = np.zeros((d, plan_u.dpad), np.float32)
        xuT[:, :plan_u.dshard] = xu[c * plan_u.dshard:(c + 1) * plan_u.dshard].T
        xiT = np.zeros((d, plan_i.dpad), np.float32)
        xiT[:, :plan_i.dshard] = xi[c * plan_i.dshard:(c + 1) * plan_i.dshard].T
        in_maps.append(dict(
            xi_bf=xi_bf, xu_bf=xu_bf, xuT=xuT, xiT=xiT,
            idx_u=idx_u, idx_i=idx_i, s_u=s_u, s_i=s_i, **consts))

    _t0 = _time.time()
    ckey = (n_u, n_i, len(u_idx), q_rows)
    if ckey in _CACHE and _CACHE[ckey][1] == (plan_u.nslots, plan_i.nslots,
                                             plan_u.s_cols, plan_i.s_cols):
        nc = _CACHE[ckey][0]
    else:
        nc = build_program(plan_u, plan_i, d)
        _CACHE[ckey] = (nc, (plan_u.nslots, plan_i.nslots,
                             plan_u.s_cols, plan_i.s_cols))
    TIMINGS["build_s"] = _time.time() - _t0
    _t0 = _time.time()
    res = run_bass_kernel_spmd(nc, in_maps, core_ids=list(range(NCORES)))
    TIMINGS["exec_s"] = _time.time() - _t0
    TIMINGS["nc"] = nc

    xu_plus = np.empty((n_u, d), np.float32)
    xi_plus = np.empty((n_i, d), np.float32)
    sq_u = ms_u = sq_i = ms_i = 0.0
    for c in range(NCORES):
        r = res.results[c]
        xu_plus[c * plan_u.dshard:(c + 1) * plan_u.dshard] = \
            r["xuT_plus"][:, :plan_u.dshard].T
        xi_plus[c * plan_i.dshard:(c + 1) * plan_i.dshard] = \
            r["xiT_plus"][:, :plan_i.dshard].T
        lp = r["loss_parts"][0]
        sq_u += float(lp[0]); ms_u += float(lp[1])
        sq_i += float(lp[2]); ms_i += float(lp[3])
    loss = np.float32(np.float32(sq_u) / np.float32(ms_u)
                      + np.float32(sq_i) / np.float32(ms_i))
    return xu_plus, xi_plus, loss


def kernel(xu_t_minus, xi_t_minus, u_idx, i_idx, vals,
           W_uu, b_uu, W_ii, b_ii, W_iu, W_ui):
    return run(xu_t_minus, xi_t_minus, u_idx, i_idx, vals,
               np.asarray(W_uu, np.float32), b_uu,
               np.asarray(W_ii, np.float32), b_ii,
               np.asarray(W_iu, np.float32), np.asarray(W_ui, np.float32),
               q_rows=25000)
